# revision 8
# baseline (speedup 1.0000x reference)
"""Multi-head attention (B=2, S=2048, H=2048, NH=16) on 8 TRN2 NeuronCores.

Sharding: tensor-parallel over heads — 2 heads per core. Each core computes
q/k/v projections for its heads, per-head attention, and a partial output
projection (its heads' columns of Wo); the host sums the 8 partials.

v2 layout (all matmuls bf16 inputs, f32 PSUM accumulation):
  - PSUM tags: "big" [128,2,1024] f32 x1 (4 banks) for score-pairs and
    even oproj tiles; "av" [128,512] x2 and "half" [128,512] x2 (1 bank
    each) for AV accumulators, denominators, qkv chunks, odd oproj tiles.
  - exp runs on ScalarE over a PAIR of score chunks per ACTIVATE (2048
    free dim) to amortize the per-instruction bubble.
  - softmax denominator: sequential DVE add chain over the 16 exp chunks
    to one [128,1024] sum, then a single deferred ones-matmul (emitted
    after the next phase has queued work, so TensorE never stalls on it).
  - oproj drains split: even token-tiles copied PSUM->SBUF on ScalarE
    ([128,2048] per copy), odd tiles on VectorE; 2 tiles in flight.
  - output rows DMA'd as full [128,2048] tiles (4KB lines), alternating
    sync/gpsimd queues; hT input streamed in 4-fc-chunk DMAs on
    sync (b=0) / gpsimd (b=1); weights in 2 chunks each, earliest-first.
"""

import sys

sys.path.insert(0, "/opt/trn_rl_repo")

from contextlib import ExitStack

import ml_dtypes
import numpy as np

import concourse.bass as bass
import concourse.tile as tile
from concourse import bacc, mybir
from concourse.bass_utils import run_bass_kernel_spmd

B, S, H, NH = 2, 2048, 2048, 16
HD = H // NH          # 128
N_CORES = 8
HPC = NH // N_CORES   # heads per core = 2
HDC = HPC * HD        # head-dims per core = 256
T = B * S             # 4096 tokens
FC = H // 128         # 16 feature chunks
TC = S // 128         # 16 token tiles per batch
SHIFT = 4.0           # fixed exp shift (softmax-invariant, overflow guard)

BF16 = mybir.dt.bfloat16
F32 = mybir.dt.float32
EXP = mybir.ActivationFunctionType.Exp
COPY = mybir.ActivationFunctionType.Copy

_CACHE = {}


def build_program(out_dtype=BF16):
    nc = bacc.Bacc(
        "TRN2", target_bir_lowering=False, debug=False, num_devices=N_CORES
    )
    hT = nc.dram_tensor("hT", [H, T], BF16, kind="ExternalInput").ap()
    wqT = nc.dram_tensor("wqT", [H, HDC], BF16, kind="ExternalInput").ap()
    wkT = nc.dram_tensor("wkT", [H, HDC], BF16, kind="ExternalInput").ap()
    wvT = nc.dram_tensor("wvT", [H, HDC], BF16, kind="ExternalInput").ap()
    woT = nc.dram_tensor("woT", [HDC, H], BF16, kind="ExternalInput").ap()
    bq = nc.dram_tensor("bq", [HDC], F32, kind="ExternalInput").ap()
    bk = nc.dram_tensor("bk", [HDC], F32, kind="ExternalInput").ap()
    bv = nc.dram_tensor("bv", [1, HDC], F32, kind="ExternalInput").ap()
    out = nc.dram_tensor("out", [T, H], out_dtype, kind="ExternalOutput").ap()

    with tile.TileContext(nc) as tc:
        _kernel(tc, out, hT, wqT, wkT, wvT, woT, bq, bk, bv)
    nc.compile()
    return nc


def _kernel(tc, out, hT, wqT, wkT, wvT, woT, bq, bk, bv):
    nc = tc.nc
    scale = 1.0 / float(np.sqrt(HD))
    ctx = ExitStack()
    with ctx:
        singles = ctx.enter_context(tc.tile_pool(name="singles", bufs=1))
        persist = ctx.enter_context(tc.tile_pool(name="persist", bufs=1))
        ps_big = ctx.enter_context(tc.tile_pool(name="ps_big", bufs=1, space="PSUM"))
        ps_av = ctx.enter_context(tc.tile_pool(name="ps_av", bufs=2, space="PSUM"))
        ps_half = ctx.enter_context(tc.tile_pool(name="ps_half", bufs=2, space="PSUM"))
        ht_pool = ctx.enter_context(tc.tile_pool(name="ht", bufs=3))
        pt_pool = ctx.enter_context(tc.tile_pool(name="pt", bufs=4))
        ts_pool = ctx.enter_context(tc.tile_pool(name="ts", bufs=6))
        rc_pool = ctx.enter_context(tc.tile_pool(name="rc", bufs=2))
        o_pool = ctx.enter_context(tc.tile_pool(name="o_sb", bufs=3))

        # ---- constants ----
        ones = singles.tile([128, 128], BF16)
        nc.vector.memset(ones, 1.0)
        neg_shift = singles.tile([128, 1], F32)
        nc.vector.memset(neg_shift, -SHIFT)

        # ---- weights: 2-chunk DMAs, earliest-consumed first ----
        w_sb = {}
        for name, ap in (("v", wvT), ("q", wqT), ("k", wkT)):
            w_sb[name] = singles.tile(
                [128, FC, HDC], BF16, tag=f"w{name}", name=f"w{name}"
            )
        for g in range(2):
            for name, ap in (("v", wvT), ("q", wqT), ("k", wkT)):
                nc.gpsimd.dma_start(
                    out=w_sb[name][:, 8 * g : 8 * g + 8, :],
                    in_=ap.rearrange("(c p) m -> p c m", p=128)[:, 8 * g : 8 * g + 8, :],
                )
        woT_sb = singles.tile([128, HPC, H], BF16)
        for g in range(2):
            nc.gpsimd.dma_start(
                out=woT_sb[:, g, :],
                in_=woT.rearrange("(h p) o -> p h o", p=128)[:, g, :],
            )
        bq_sb = singles.tile([128, HPC], F32)
        nc.scalar.dma_start(out=bq_sb, in_=bq.rearrange("(h p) -> p h", p=128))
        bk_sb = singles.tile([128, HPC], F32)
        nc.scalar.dma_start(out=bk_sb, in_=bk.rearrange("(h p) -> p h", p=128))
        # bv broadcast to [128, 2, 256] (stride-0 partition and group dims)
        bv2 = singles.tile([128, 2, HDC], F32)
        nc.scalar.dma_start(
            out=bv2,
            in_=bass.AP(tensor=bv.tensor, offset=bv.offset,
                        ap=[[0, 128], [0, 2], [1, HDC]]),
        )

        # ---- persistent activations ----
        qt_sb = [[persist.tile([128, S], BF16, tag=f"qt{b}{h}", name=f"qt{b}{h}")
                  for h in range(HPC)] for b in range(B)]
        kt_sb = [[persist.tile([128, S], BF16, tag=f"kt{b}{h}", name=f"kt{b}{h}")
                  for h in range(HPC)] for b in range(B)]
        v_sb = [persist.tile([128, TC, HDC], BF16, tag=f"v{b}", name=f"v{b}")
                for b in range(B)]
        aoT_sb = [[persist.tile([128, S], BF16, tag=f"ao{b}{h}", name=f"ao{b}{h}")
                   for h in range(HPC)] for b in range(B)]

        hT_re = hT.rearrange("(c p) t -> p c t", p=128)

        # hT tiles: [128, FC, 512], one per 512-token block; 4-fc-chunk DMAs.
        ht_tiles = {}

        def load_ht(b, blk):
            key = (b, blk)
            if key in ht_tiles:
                return ht_tiles[key]
            t0 = b * S + blk * 512
            t = ht_pool.tile([128, FC, 512], BF16, tag="ht", name=f"ht{b}{blk}")
            eng = nc.sync if b == 0 else nc.gpsimd
            for g in range(4):
                eng.dma_start(
                    out=t[:, 4 * g : 4 * g + 4, :],
                    in_=hT_re[:, 4 * g : 4 * g + 4, t0 : t0 + 512],
                )
            ht_tiles[key] = t
            return t

        # deferred softmax-denominator closures (den matmul + recip + norm)
        pending = []

        def flush_pending():
            while pending:
                pending.pop(0)()

        def v_block(b, ht_t, blk, s2):
            # two 128-token sub-blocks -> v_sb[:, blk*4 + 2*s2 : +2, :]
            ps = ps_half.tile([128, 2, HDC], F32, tag="half",
                              name=f"v{b}{blk}{s2}")
            for s in range(2):
                col = (2 * s2 + s) * 128
                for fc in range(FC):
                    nc.tensor.matmul(
                        ps[:, s, :],
                        ht_t[:, fc, col : col + 128],
                        w_sb["v"][:, fc, :],
                        start=(fc == 0),
                        stop=(fc == FC - 1),
                    )
            tt0 = blk * 4 + s2 * 2
            nc.vector.tensor_add(v_sb[b][:, tt0 : tt0 + 2, :], ps, bv2)

        def qk_block(b, ht_t, blk, h, name, dst, bias):
            ps = ps_half.tile([128, 512], F32, tag="half",
                              name=f"qk{b}{blk}{h}{name}")
            for fc in range(FC):
                nc.tensor.matmul(
                    ps,
                    w_sb[name][:, fc, h * HD : (h + 1) * HD],
                    ht_t[:, fc, :],
                    start=(fc == 0),
                    stop=(fc == FC - 1),
                )
            nc.vector.tensor_scalar_add(
                dst[:, blk * 512 : (blk + 1) * 512], ps, bias[:, h : h + 1],
            )

        def qkv_half(b, half):
            for qx in range(2):
                blk = half * 2 + qx
                ht_t = load_ht(b, blk)
                if blk + 1 < 4:
                    load_ht(b, blk + 1)  # prefetch next block's DMA
                elif b == 0:
                    load_ht(1, 0)
                v_block(b, ht_t, blk, 0)
                for h in range(HPC):
                    qk_block(b, ht_t, blk, h, "q", qt_sb[b][h], bq_sb)
                    qk_block(b, ht_t, blk, h, "k", kt_sb[b][h], bk_sb)
                v_block(b, ht_t, blk, 1)

        def attention(b, qh):
            q0 = qh * 1024
            for h in range(HPC):
                # flush the previous group's den/normalize BEFORE reusing
                # the ps_av slots it still holds (TensorE is in-order: a
                # later-emitted den matmul cannot unblock an earlier wait).
                flush_pending()
                av = [ps_av.tile([128, 512], F32, tag="av",
                                 name=f"av{b}{h}{qh}{n}") for n in range(2)]
                t16_prev = None
                acc = None
                for p in range(TC // 2):
                    ps = ps_big.tile([128, 2, 1024], F32, tag="big",
                                     name=f"sc{b}{h}{qh}{p}")
                    for j in range(2):
                        tcx = 2 * p + j
                        lhsT = kt_sb[b][h][:, tcx * 128 : (tcx + 1) * 128]
                        for n in range(2):
                            nc.tensor.matmul(
                                ps[:, j, n * 512 : (n + 1) * 512],
                                lhsT,
                                qt_sb[b][h][:, q0 + n * 512 : q0 + (n + 1) * 512],
                                start=True,
                                stop=True,
                            )
                    pt = pt_pool.tile([128, 2, 1024], BF16, tag="pt",
                                      name=f"pt{b}{h}{qh}{p}")
                    nc.scalar.activation(pt, ps, EXP,
                                         bias=neg_shift, scale=scale)
                    for j in range(2):
                        tcx = 2 * p + j
                        for n in range(2):
                            nc.tensor.matmul(
                                av[n],
                                v_sb[b][:, tcx, h * HD : (h + 1) * HD],
                                pt[:, j, n * 512 : (n + 1) * 512],
                                start=(tcx == 0),
                                stop=(tcx == TC - 1),
                            )
                    # denominator partials: shallow tree (pair chunks ->
                    # t16 pairs -> p2 -> chain of 4) keeps bf16 rounding
                    # depth ~5 while staying 15 adds total per group.
                    t16 = ts_pool.tile([128, 1024], BF16, tag="ts",
                                       name=f"t16{b}{h}{qh}{p}")
                    nc.vector.tensor_add(t16, pt[:, 0, :], pt[:, 1, :])
                    if t16_prev is None:
                        t16_prev = t16
                    else:
                        p2 = ts_pool.tile([128, 1024], BF16, tag="ts",
                                          name=f"p2{b}{h}{qh}{p}")
                        nc.vector.tensor_add(p2, t16_prev, t16)
                        t16_prev = None
                        if acc is None:
                            acc = p2
                        else:
                            nacc = ts_pool.tile([128, 1024], BF16, tag="ts",
                                                name=f"acc{b}{h}{qh}{p}")
                            nc.vector.tensor_add(nacc, acc, p2)
                            acc = nacc

                def den_norm(b=b, h=h, q0=q0, av=av, acc=acc):
                    rc = rc_pool.tile([128, 1024], F32, tag="rc",
                                      name=f"rc{b}{h}{q0}")
                    for n in range(2):
                        den = ps_half.tile([128, 512], F32, tag="half",
                                           name=f"den{b}{h}{q0}{n}")
                        nc.tensor.matmul(
                            den, ones, acc[:, n * 512 : (n + 1) * 512],
                            start=True, stop=True,
                        )
                        nc.vector.reciprocal_approx_fast(
                            rc[:, n * 512 : (n + 1) * 512], den)
                        nc.vector.tensor_mul(
                            aoT_sb[b][h][:, q0 + n * 512 : q0 + (n + 1) * 512],
                            av[n], rc[:, n * 512 : (n + 1) * 512])

                pending.append(den_norm)

        def oproj(b, tts):
            for tt in tts:
                if tt == tts[0]:
                    flush_pending()
                o_t = o_pool.tile([128, H], BF16, tag="o", name=f"ot{b}{tt}")
                if tt % 2 == 0:
                    ps = ps_big.tile([128, H], F32, tag="big",
                                     name=f"op{b}{tt}")
                    for h in range(HPC):
                        lhsT = aoT_sb[b][h][:, tt * 128 : (tt + 1) * 128]
                        for oc in range(4):
                            nc.tensor.matmul(
                                ps[:, oc * 512 : (oc + 1) * 512],
                                lhsT,
                                woT_sb[:, h, oc * 512 : (oc + 1) * 512],
                                start=(h == 0),
                                stop=(h == HPC - 1),
                            )
                    nc.scalar.activation(o_t, ps, COPY)
                else:
                    for oc in range(4):
                        pool = ps_half if oc < 2 else ps_av
                        ps = pool.tile([128, 512], F32,
                                       tag="half" if oc < 2 else "av",
                                       name=f"op{b}{tt}{oc}")
                        for h in range(HPC):
                            lhsT = aoT_sb[b][h][:, tt * 128 : (tt + 1) * 128]
                            nc.tensor.matmul(
                                ps,
                                lhsT,
                                woT_sb[:, h, oc * 512 : (oc + 1) * 512],
                                start=(h == 0),
                                stop=(h == HPC - 1),
                            )
                        nc.vector.tensor_copy(
                            o_t[:, oc * 512 : (oc + 1) * 512], ps)
                row0 = b * S + tt * 128
                eng = nc.sync if tt % 2 == 0 else nc.gpsimd
                eng.dma_start(out=out[row0 : row0 + 128, :], in_=o_t)

        qkv_half(0, 0)
        qkv_half(0, 1)
        attention(0, 0)
        qkv_half(1, 0)
        attention(0, 1)
        qkv_half(1, 1)
        attention(1, 0)
        attention(1, 1)
        oproj(0, list(range(0, TC)))
        oproj(1, list(range(0, TC)))
        flush_pending()


def kernel(hidden_state, Wq, bq, Wk, bk, Wv, bv, Wo, bo):
    bf16 = ml_dtypes.bfloat16
    h2 = np.asarray(hidden_state, dtype=np.float32).reshape(T, H)
    hT = np.ascontiguousarray(h2.T).astype(bf16)

    in_maps = []
    for c in range(N_CORES):
        r0 = c * HDC
        in_maps.append({
            "hT": hT,
            "wqT": np.ascontiguousarray(
                np.asarray(Wq, np.float32)[r0 : r0 + HDC, :].T).astype(bf16),
            "wkT": np.ascontiguousarray(
                np.asarray(Wk, np.float32)[r0 : r0 + HDC, :].T).astype(bf16),
            "wvT": np.ascontiguousarray(
                np.asarray(Wv, np.float32)[r0 : r0 + HDC, :].T).astype(bf16),
            "woT": np.ascontiguousarray(
                np.asarray(Wo, np.float32)[:, r0 : r0 + HDC].T).astype(bf16),
            "bq": np.asarray(bq, np.float32)[r0 : r0 + HDC].copy(),
            "bk": np.asarray(bk, np.float32)[r0 : r0 + HDC].copy(),
            "bv": np.asarray(bv, np.float32)[r0 : r0 + HDC].reshape(1, HDC).copy(),
        })

    if "nc" not in _CACHE:
        _CACHE["nc"] = build_program()
    nc = _CACHE["nc"]
    _CACHE["in_maps"] = in_maps

    res = run_bass_kernel_spmd(nc, in_maps, core_ids=list(range(N_CORES)))
    total = np.zeros((T, H), np.float32)
    for r in res.results:
        total += np.asarray(r["out"]).astype(np.float32)
    total += np.asarray(bo, np.float32)[None, :]
    return total.reshape(B, S, H)


# revision 13
# speedup vs baseline: 1.3241x; 1.3241x over previous
"""Multi-head attention (B=2, S=2048, H=2048, NH=16) on 8 TRN2 NeuronCores.

Sharding: tensor-parallel over heads — 2 heads per core. Each core computes
q/k/v projections for its heads, per-head attention, and a partial output
projection (its heads' columns of Wo); the host sums the 8 partials.

v3: software-pipelined emission. Attention is ScalarE-bound (exp); qkv and
oproj matmul units are emitted as "fillers" between attention chunks so the
in-order TensorE queue always has ready work during exp waits.
  - PSUM: "sc" [128,1024] x2 (scores, 4 banks), "av" [128,512] x2 (AV
    accumulators, 2 banks), "half" [128,512] x2 (den / qkv chunks / filler
    oproj / fast oproj, 2 banks).
  - softmax denominator: shallow DVE add tree (depth ~5) to one [128,1024]
    bf16 sum + single deferred ones-matmul per (b,h,qh) group, flushed at
    the NEXT group's start (before its ps_av allocations — ordering matters:
    TensorE is in-order, so the den matmul must precede any matmul that
    waits on the slots it frees).
  - oproj fast path: 2 token-tiles in flight (sc slots / av+half slots);
    drains split between ScalarE ([128,1024] copies) and VectorE.
  - output rows DMA'd as [128,2048] tiles (4KB lines) on sync/gpsimd;
    hT streamed in 4-fc-chunk DMAs (sync for b=0, gpsimd for b=1);
    weights 2 chunks each, earliest-consumed first.
"""

import sys

sys.path.insert(0, "/opt/trn_rl_repo")

from contextlib import ExitStack

import ml_dtypes
import numpy as np

import concourse.bass as bass
import concourse.tile as tile
from concourse import bacc, mybir
from concourse.bass_utils import run_bass_kernel_spmd

B, S, H, NH = 2, 2048, 2048, 16
HD = H // NH          # 128
N_CORES = 8
HPC = NH // N_CORES   # heads per core = 2
HDC = HPC * HD        # head-dims per core = 256
T = B * S             # 4096 tokens
FC = H // 128         # 16 feature chunks
TC = S // 128         # 16 token tiles per batch
SHIFT = 4.0           # fixed exp shift (softmax-invariant, overflow guard)

BF16 = mybir.dt.bfloat16
F32 = mybir.dt.float32
EXP = mybir.ActivationFunctionType.Exp
COPY = mybir.ActivationFunctionType.Copy

_CACHE = {}


def build_program(out_dtype=BF16):
    nc = bacc.Bacc(
        "TRN2", target_bir_lowering=False, debug=False, num_devices=N_CORES
    )
    hT = nc.dram_tensor("hT", [H, T], BF16, kind="ExternalInput").ap()
    wqT = nc.dram_tensor("wqT", [H, HDC], BF16, kind="ExternalInput").ap()
    wkT = nc.dram_tensor("wkT", [H, HDC], BF16, kind="ExternalInput").ap()
    wvT = nc.dram_tensor("wvT", [H, HDC], BF16, kind="ExternalInput").ap()
    woT = nc.dram_tensor("woT", [HDC, H], BF16, kind="ExternalInput").ap()
    bq = nc.dram_tensor("bq", [HDC], F32, kind="ExternalInput").ap()
    bk = nc.dram_tensor("bk", [HDC], F32, kind="ExternalInput").ap()
    bv = nc.dram_tensor("bv", [1, HDC], F32, kind="ExternalInput").ap()
    out = nc.dram_tensor("out", [T, H], out_dtype, kind="ExternalOutput").ap()

    with tile.TileContext(nc) as tc:
        _kernel(tc, out, hT, wqT, wkT, wvT, woT, bq, bk, bv)
    nc.compile()
    return nc


def _kernel(tc, out, hT, wqT, wkT, wvT, woT, bq, bk, bv):
    nc = tc.nc
    scale = 1.0 / float(np.sqrt(HD))
    ctx = ExitStack()
    with ctx:
        singles = ctx.enter_context(tc.tile_pool(name="singles", bufs=1))
        persist = ctx.enter_context(tc.tile_pool(name="persist", bufs=1))
        ps_sc = ctx.enter_context(tc.tile_pool(name="ps_sc", bufs=2, space="PSUM"))
        ps_av = ctx.enter_context(tc.tile_pool(name="ps_av", bufs=2, space="PSUM"))
        ps_half = ctx.enter_context(tc.tile_pool(name="ps_half", bufs=2, space="PSUM"))
        ht_pool = ctx.enter_context(tc.tile_pool(name="ht", bufs=3))
        pt_pool = ctx.enter_context(tc.tile_pool(name="pt", bufs=5))
        ts_pool = ctx.enter_context(tc.tile_pool(name="ts", bufs=6))
        rc_pool = ctx.enter_context(tc.tile_pool(name="rc", bufs=2))
        o_pool = ctx.enter_context(tc.tile_pool(name="o_sb", bufs=4))

        # ---- constants ----
        ones = singles.tile([128, 128], BF16)
        nc.vector.memset(ones, 1.0)
        neg_shift = singles.tile([128, 1], F32)
        nc.vector.memset(neg_shift, -SHIFT)

        # ---- weights: 2-chunk DMAs, earliest-consumed first ----
        w_sb = {}
        for name in ("v", "q", "k"):
            w_sb[name] = singles.tile(
                [128, FC, HDC], BF16, tag=f"w{name}", name=f"w{name}"
            )
        for g in range(2):
            for name, ap in (("v", wvT), ("q", wqT), ("k", wkT)):
                nc.gpsimd.dma_start(
                    out=w_sb[name][:, 8 * g : 8 * g + 8, :],
                    in_=ap.rearrange("(c p) m -> p c m", p=128)[:, 8 * g : 8 * g + 8, :],
                )
        woT_sb = singles.tile([128, HPC, H], BF16)
        for g in range(2):
            nc.gpsimd.dma_start(
                out=woT_sb[:, g, :],
                in_=woT.rearrange("(h p) o -> p h o", p=128)[:, g, :],
            )
        bq_sb = singles.tile([128, HPC], F32)
        nc.scalar.dma_start(out=bq_sb, in_=bq.rearrange("(h p) -> p h", p=128))
        bk_sb = singles.tile([128, HPC], F32)
        nc.scalar.dma_start(out=bk_sb, in_=bk.rearrange("(h p) -> p h", p=128))
        bv2 = singles.tile([128, 2, HDC], F32)
        nc.scalar.dma_start(
            out=bv2,
            in_=bass.AP(tensor=bv.tensor, offset=bv.offset,
                        ap=[[0, 128], [0, 2], [1, HDC]]),
        )

        # ---- persistent activations ----
        qt_sb = [[persist.tile([128, S], BF16, tag=f"qt{b}{h}", name=f"qt{b}{h}")
                  for h in range(HPC)] for b in range(B)]
        kt_sb = [[persist.tile([128, S], BF16, tag=f"kt{b}{h}", name=f"kt{b}{h}")
                  for h in range(HPC)] for b in range(B)]
        v_sb = [persist.tile([128, TC, HDC], BF16, tag=f"v{b}", name=f"v{b}")
                for b in range(B)]
        aoT_sb = [[persist.tile([128, S], BF16, tag=f"ao{b}{h}", name=f"ao{b}{h}")
                   for h in range(HPC)] for b in range(B)]

        hT_re = hT.rearrange("(c p) t -> p c t", p=128)
        ht_tiles = {}

        def load_ht(b, blk):
            key = (b, blk)
            if key in ht_tiles:
                return ht_tiles[key]
            t0 = b * S + blk * 512
            t = ht_pool.tile([128, FC, 512], BF16, tag="ht", name=f"ht{b}{blk}")
            eng = nc.sync if b == 0 else nc.gpsimd
            for g in range(4):
                eng.dma_start(
                    out=t[:, 4 * g : 4 * g + 4, :],
                    in_=hT_re[:, 4 * g : 4 * g + 4, t0 : t0 + 512],
                )
            ht_tiles[key] = t
            return t

        # ---- deferred softmax denominator ----
        pending = []

        def flush_pending():
            while pending:
                pending.pop(0)()

        # ---- filler queue (units of TensorE work to hide exp waits) ----
        filler_q = []

        def fill(n):
            while n > 0 and filler_q:
                try:
                    next(filler_q[0])
                    n -= 1
                except StopIteration:
                    filler_q.pop(0)

        def drain_fillers():
            while filler_q:
                fill(1)

        # ---- qkv projection units ----
        def v_block(b, ht_t, blk, s2):
            ps = ps_half.tile([128, 2, HDC], F32, tag="half",
                              name=f"v{b}{blk}{s2}")
            for s in range(2):
                col = (2 * s2 + s) * 128
                for fc in range(FC):
                    nc.tensor.matmul(
                        ps[:, s, :],
                        ht_t[:, fc, col : col + 128],
                        w_sb["v"][:, fc, :],
                        start=(fc == 0),
                        stop=(fc == FC - 1),
                    )
            tt0 = blk * 4 + s2 * 2
            nc.vector.tensor_add(v_sb[b][:, tt0 : tt0 + 2, :], ps, bv2)

        def qk_block(b, ht_t, blk, h, name, dst, bias):
            ps = ps_half.tile([128, 512], F32, tag="half",
                              name=f"qk{b}{blk}{h}{name}")
            for fc in range(FC):
                nc.tensor.matmul(
                    ps,
                    w_sb[name][:, fc, h * HD : (h + 1) * HD],
                    ht_t[:, fc, :],
                    start=(fc == 0),
                    stop=(fc == FC - 1),
                )
            nc.vector.tensor_scalar_add(
                dst[:, blk * 512 : (blk + 1) * 512], ps, bias[:, h : h + 1],
            )

        def qkv_gen(b, half):
            for qx in range(2):
                blk = half * 2 + qx
                ht_t = load_ht(b, blk)
                if blk + 1 < 4:
                    load_ht(b, blk + 1)
                elif b == 0:
                    load_ht(1, 0)
                v_block(b, ht_t, blk, 0)
                yield
                for h in range(HPC):
                    qk_block(b, ht_t, blk, h, "q", qt_sb[b][h], bq_sb)
                    yield
                    qk_block(b, ht_t, blk, h, "k", kt_sb[b][h], bk_sb)
                    yield
                v_block(b, ht_t, blk, 1)
                yield

        def qkv_half(b, half):
            for _ in qkv_gen(b, half):
                pass

        # ---- attention ----
        def attention(b, qh, fill_every=2):
            q0 = qh * 1024
            for h in range(HPC):
                # flush previous group's den/normalize BEFORE reusing the
                # ps_av slots it still holds (TensorE is in-order).
                flush_pending()
                av = [ps_av.tile([128, 512], F32, tag="av",
                                 name=f"av{b}{h}{qh}{n}") for n in range(2)]
                pt_prev = None
                t2_prev = None
                acc = None
                for tcx in range(TC):
                    ps = ps_sc.tile([128, 1024], F32, tag="sc",
                                    name=f"sc{b}{h}{qh}{tcx}")
                    lhsT = kt_sb[b][h][:, tcx * 128 : (tcx + 1) * 128]
                    for n in range(2):
                        nc.tensor.matmul(
                            ps[:, n * 512 : (n + 1) * 512],
                            lhsT,
                            qt_sb[b][h][:, q0 + n * 512 : q0 + (n + 1) * 512],
                            start=True,
                            stop=True,
                        )
                    pt = pt_pool.tile([128, 1024], BF16, tag="pt",
                                      name=f"pt{b}{h}{qh}{tcx}")
                    nc.scalar.activation(pt, ps, EXP,
                                         bias=neg_shift, scale=scale)
                    for n in range(2):
                        nc.tensor.matmul(
                            av[n],
                            v_sb[b][:, tcx, h * HD : (h + 1) * HD],
                            pt[:, n * 512 : (n + 1) * 512],
                            start=(tcx == 0),
                            stop=(tcx == TC - 1),
                        )
                    if tcx % fill_every == fill_every - 1:
                        fill(1)
                    # denominator tree: pt pairs -> t2, t2 pairs -> p4,
                    # chain p4s (depth ~5 in bf16)
                    if pt_prev is None:
                        pt_prev = pt
                    else:
                        t2 = ts_pool.tile([128, 1024], BF16, tag="ts",
                                          name=f"t2{b}{h}{qh}{tcx}")
                        nc.vector.tensor_add(t2, pt_prev, pt)
                        pt_prev = None
                        if t2_prev is None:
                            t2_prev = t2
                        else:
                            p4 = ts_pool.tile([128, 1024], BF16, tag="ts",
                                              name=f"p4{b}{h}{qh}{tcx}")
                            nc.vector.tensor_add(p4, t2_prev, t2)
                            t2_prev = None
                            if acc is None:
                                acc = p4
                            else:
                                nacc = ts_pool.tile(
                                    [128, 1024], BF16, tag="ts",
                                    name=f"acc{b}{h}{qh}{tcx}")
                                nc.vector.tensor_add(nacc, acc, p4)
                                acc = nacc

                def den_norm(b=b, h=h, q0=q0, av=av, acc=acc):
                    rc = rc_pool.tile([128, 1024], F32, tag="rc",
                                      name=f"rc{b}{h}{q0}")
                    for n in range(2):
                        den = ps_half.tile([128, 512], F32, tag="half",
                                           name=f"den{b}{h}{q0}{n}")
                        nc.tensor.matmul(
                            den, ones, acc[:, n * 512 : (n + 1) * 512],
                            start=True, stop=True,
                        )
                        nc.vector.reciprocal_approx_fast(
                            rc[:, n * 512 : (n + 1) * 512], den)
                        nc.vector.tensor_mul(
                            aoT_sb[b][h][:, q0 + n * 512 : q0 + (n + 1) * 512],
                            av[n], rc[:, n * 512 : (n + 1) * 512])

                pending.append(den_norm)

        # ---- output projection ----
        def oproj_mm(b, tt, pss):
            # pss: 4 (psum_tile, col0) pairs covering [128, 2048]
            for h in range(HPC):
                lhsT = aoT_sb[b][h][:, tt * 128 : (tt + 1) * 128]
                for ps, oc0, w in pss:
                    nc.tensor.matmul(
                        ps,
                        lhsT,
                        woT_sb[:, h, oc0 : oc0 + w],
                        start=(h == 0),
                        stop=(h == HPC - 1),
                    )

        def oproj_gen(b, tts):
            # attention-safe filler units: only ps_half slots, DVE drains
            for tt in tts:
                o_t = o_pool.tile([128, H], BF16, tag="o", name=f"ot{b}{tt}")
                for ocp in range(2):
                    for oc in (2 * ocp, 2 * ocp + 1):
                        ps = ps_half.tile([128, 512], F32, tag="half",
                                          name=f"op{b}{tt}{oc}")
                        for h in range(HPC):
                            nc.tensor.matmul(
                                ps,
                                aoT_sb[b][h][:, tt * 128 : (tt + 1) * 128],
                                woT_sb[:, h, oc * 512 : (oc + 1) * 512],
                                start=(h == 0),
                                stop=(h == HPC - 1),
                            )
                        nc.vector.tensor_copy(
                            o_t[:, oc * 512 : (oc + 1) * 512], ps)
                    if ocp == 1:
                        row0 = b * S + tt * 128
                        eng = nc.sync if tt % 2 == 0 else nc.gpsimd
                        eng.dma_start(out=out[row0 : row0 + 128, :], in_=o_t)
                    yield

        def oproj_fast(b, tts):
            # post-attention path: 2 token-tiles in flight, drains split
            # between ScalarE (sc-slot tiles) and VectorE (av+half tiles)
            for tt in tts:
                o_t = o_pool.tile([128, H], BF16, tag="o", name=f"ot{b}{tt}")
                if tt % 2 == 0:
                    tiles = [ps_sc.tile([128, 1024], F32, tag="sc",
                                        name=f"op{b}{tt}{k}") for k in range(2)]
                    pss = []
                    for k in range(2):
                        pss.append((tiles[k][:, 0:512], k * 1024, 512))
                        pss.append((tiles[k][:, 512:1024], k * 1024 + 512, 512))
                    oproj_mm(b, tt, pss)
                    for k in range(2):
                        nc.scalar.activation(
                            o_t[:, k * 1024 : (k + 1) * 1024], tiles[k], COPY)
                else:
                    pss = []
                    for oc in range(4):
                        pool, tag = (ps_av, "av") if oc >= 2 else (ps_half, "half")
                        ps = pool.tile([128, 512], F32, tag=tag,
                                       name=f"op{b}{tt}{oc}")
                        pss.append((ps, oc * 512, 512))
                    oproj_mm(b, tt, pss)
                    for oc in range(4):
                        nc.vector.tensor_copy(
                            o_t[:, oc * 512 : (oc + 1) * 512], pss[oc][0])
                row0 = b * S + tt * 128
                eng = nc.sync if tt % 2 == 0 else nc.gpsimd
                eng.dma_start(out=out[row0 : row0 + 128, :], in_=o_t)

        # ---- main schedule ----
        qkv_half(0, 0)
        qkv_half(0, 1)
        filler_q.append(qkv_gen(1, 0))
        attention(0, 0)
        filler_q.append(qkv_gen(1, 1))
        attention(0, 1)
        drain_fillers()          # qkv(1,*) must complete before att(1,*)
        filler_q.append(oproj_gen(0, list(range(0, TC))))
        attention(1, 0, fill_every=4)
        attention(1, 1, fill_every=4)
        drain_fillers()
        flush_pending()
        oproj_fast(1, list(range(0, TC)))
        flush_pending()


def kernel(hidden_state, Wq, bq, Wk, bk, Wv, bv, Wo, bo):
    bf16 = ml_dtypes.bfloat16
    h2 = np.asarray(hidden_state, dtype=np.float32).reshape(T, H)
    hT = np.ascontiguousarray(h2.T).astype(bf16)

    in_maps = []
    for c in range(N_CORES):
        r0 = c * HDC
        in_maps.append({
            "hT": hT,
            "wqT": np.ascontiguousarray(
                np.asarray(Wq, np.float32)[r0 : r0 + HDC, :].T).astype(bf16),
            "wkT": np.ascontiguousarray(
                np.asarray(Wk, np.float32)[r0 : r0 + HDC, :].T).astype(bf16),
            "wvT": np.ascontiguousarray(
                np.asarray(Wv, np.float32)[r0 : r0 + HDC, :].T).astype(bf16),
            "woT": np.ascontiguousarray(
                np.asarray(Wo, np.float32)[:, r0 : r0 + HDC].T).astype(bf16),
            "bq": np.asarray(bq, np.float32)[r0 : r0 + HDC].copy(),
            "bk": np.asarray(bk, np.float32)[r0 : r0 + HDC].copy(),
            "bv": np.asarray(bv, np.float32)[r0 : r0 + HDC].reshape(1, HDC).copy(),
        })

    if "nc" not in _CACHE:
        _CACHE["nc"] = build_program()
    nc = _CACHE["nc"]
    _CACHE["in_maps"] = in_maps

    res = run_bass_kernel_spmd(nc, in_maps, core_ids=list(range(N_CORES)))
    total = np.zeros((T, H), np.float32)
    for r in res.results:
        total += np.asarray(r["out"]).astype(np.float32)
    total += np.asarray(bo, np.float32)[None, :]
    return total.reshape(B, S, H)
